# revision 9
# baseline (speedup 1.0000x reference)
"""Distributed Trainium2 Bass kernel for 16-head attention.

Reference op: B=2, S=2048, D=1024, H=16 multi-head attention with an
elementwise 0/1 mask, computed as
    out = softmax(mask((q Wq^T)(k Wk^T)^T / sqrt(64))) (v Wv^T) Wo^T

Sharding over 8 NeuronCores: core c handles batch c//4 and head group
c%4 (4 heads = 256 channels). Attention is computed fully locally in a
"dual" layout (scores transposed, [k, q]); the context is exchanged
with one small AllGather per (qtile, head-pair) inside each 4-core
batch group, and the output projection is split along the OUTPUT
feature dim (each core holds a 256-column slice of Wo^T), so the
host-side unshard is a pure concatenation.

Structure: 64 slots of (scores -> exp -> mask -> ctx), ordered
pair-outer (u = 4*pair + t) so the pair-1 Q/K projections become
mid-loop PE filler. A PE warmup block + exp-table preload run during
the input DMA so the HAM clock gate is released before real work.
Out-projection matmuls drain 3+ slots after their AllGather readback
so they never head-of-line-block the PE queue. Mask arrives
host-pre-tiled so each tile is 4 contiguous DMAs.

Compute dtype bf16 (TensorE 1 cyc/row), accumulation f32 in PSUM.
"""

import sys

sys.path.insert(0, "/opt/trn_rl_repo")

import numpy as np
import ml_dtypes

BF16 = ml_dtypes.bfloat16

B = 2
S = 2048
DM = 1024
DL = 256  # d-model slice per core (4 heads)
HL = 4  # heads per core
DK = 64
P = 128
QT_N = 4  # query tiles of 512
QTS = 512
KC = 16  # key chunks of 128
MC = 8  # contraction chunks of 128 over d_model
GROUPS = [[0, 1, 2, 3], [4, 5, 6, 7]]

# slot order: u = 4*pair + t
U_LIST = [(0, 0), (0, 1), (0, 2), (0, 3), (1, 0), (1, 1), (1, 2), (1, 3)]

_cached = {}


def _build():
    import concourse.bass as bass
    import concourse.mybir as mybir
    from concourse import bacc
    from concourse.tile import TileContext

    fp32 = mybir.dt.float32
    bf16 = mybir.dt.bfloat16

    nc = bacc.Bacc(num_devices=8)

    qT = nc.dram_tensor("qT", [DM, S], bf16, kind="ExternalInput")
    kT = nc.dram_tensor("kT", [DM, S], bf16, kind="ExternalInput")
    vT = nc.dram_tensor("vT", [DM, S], bf16, kind="ExternalInput")
    # mask pre-tiled on host: maskt[128*t + p, kc*512 + q] = mask[b][512t+q, 128kc+p]
    maskt = nc.dram_tensor("maskt", [QT_N * P, KC * QTS], bf16, kind="ExternalInput")
    wq = nc.dram_tensor("wq", [DM, DL], bf16, kind="ExternalInput")
    wk = nc.dram_tensor("wk", [DM, DL], bf16, kind="ExternalInput")
    wv = nc.dram_tensor("wv", [DM, DL], bf16, kind="ExternalInput")
    wo = nc.dram_tensor("wo", [DM, DL], bf16, kind="ExternalInput")
    y = nc.dram_tensor("y", [S, DL], fp32, kind="ExternalOutput")

    cc_in = [
        [
            nc.dram_tensor(f"cc_in{t}_{p}", [P, QTS], bf16, kind="Internal")
            for p in range(2)
        ]
        for t in range(QT_N)
    ]
    cc_out = [
        [
            nc.dram_tensor(f"cc_out{t}_{p}", [4 * P, QTS], bf16, kind="Internal")
            for p in range(2)
        ]
        for t in range(QT_N)
    ]

    with TileContext(nc) as tc:
        with (
            tc.tile_pool(name="stage", bufs=16) as stage_pool,  # kT/qT/vT slices
            tc.tile_pool(name="w", bufs=32) as w_pool,
            tc.tile_pool(name="qkt", bufs=2) as qkt_pool,  # QT/KT [128,2048]
            tc.tile_pool(name="vext", bufs=16) as vext_pool,
            tc.tile_pool(name="mask", bufs=4) as mask_pool,
            tc.tile_pool(name="attn", bufs=4) as attn_pool,
            tc.tile_pool(name="sm", bufs=3) as sm_pool,
            tc.tile_pool(name="ctxn", bufs=8) as ctxn_pool,
            tc.tile_pool(name="ctxg", bufs=2) as ctxg_pool,
            tc.tile_pool(name="ysb", bufs=2) as y_pool,
            tc.tile_pool(name="ps_big", bufs=2, space="PSUM") as ps_big,
            tc.tile_pool(name="ps_acc", bufs=2, space="PSUM") as ps_acc,
            tc.tile_pool(name="ps_out", bufs=2, space="PSUM") as ps_out,
        ):
            # ---- PE warmup + ACT table preload (runs during input DMA) ------
            junk = sm_pool.tile([P, QTS], bf16, tag="junk", bufs=1)
            nc.vector.memset(junk[:], 0.25)
            warm_ps = ps_big.tile([P, 1024], fp32, tag="big", name="warm")
            for _ in range(20):
                nc.tensor.matmul(
                    warm_ps[:, 0:QTS], junk[:, 0:P], junk[:],
                    start=True, stop=True,
                )
            junk2 = sm_pool.tile([P, 16], bf16, tag="junk2", bufs=1)
            nc.scalar.activation(
                junk2[:], warm_ps[:, 0:16], mybir.ActivationFunctionType.Exp
            )

            # ---- helpers -----------------------------------------------------
            def load_w(dram, nm):
                tiles = []
                for m in range(MC):
                    t_ = w_pool.tile([P, DL], bf16, tag="w", name=f"w{nm}{m}")
                    nc.sync.dma_start(t_[:], dram[P * m : P * (m + 1), :])
                    tiles.append(t_)
                return tiles

            STAGE_BUFS = {"kT": 12, "qT": 16, "vT": 8}

            def load_slice(dram, m, st, tag, nm):
                t_ = stage_pool.tile(
                    [P, QTS], bf16, tag=tag, bufs=STAGE_BUFS[tag], name=f"x{nm}"
                )
                nc.sync.dma_start(
                    t_[:], dram[P * m : P * (m + 1), QTS * st : QTS * (st + 1)]
                )
                return t_

            # one unit: out_sb[:, 512st:...] = (w col slice).T @ x_st
            def proj_unit(x_slices, w_sb, wcol, out_sb, st, nm):
                ps = ps_out.tile([P, QTS], fp32, tag="out", name=f"pp{nm}")
                for m in range(MC):
                    nc.tensor.matmul(
                        ps[:],
                        w_sb[m][:, P * wcol : P * (wcol + 1)],
                        x_slices[m][:],
                        start=(m == 0),
                        stop=(m == MC - 1),
                    )
                nc.vector.tensor_copy(out_sb[:, QTS * st : QTS * (st + 1)], ps[:])

            def v_unit(vT_slices, soff, wv_sb, st):
                ps = ps_out.tile([P, DL], fp32, tag="out", name=f"vp{st}")
                for m in range(MC):
                    nc.tensor.matmul(
                        ps[:],
                        vT_slices[m][:, P * soff : P * (soff + 1)],
                        wv_sb[m][:],
                        start=(m == 0),
                        stop=(m == MC - 1),
                    )
                ve = vext_pool.tile(
                    [P, HL * (DK + 1)], bf16, tag="vext", name=f"ve{st}"
                )
                nc.vector.memset(
                    ve[:].rearrange("p (h d) -> p h d", d=DK + 1)[:, :, DK : DK + 1],
                    1.0,
                )
                dst3 = ve[:].rearrange("p (h d) -> p h d", d=DK + 1)[:, :, 0:DK]
                src3 = ps[:].rearrange("p (h d) -> p h d", d=DK)
                nc.vector.tensor_copy(dst3, src3)
                return ve

            def load_mask(t):
                return mask_pool.tile(
                    [P, KC * QTS], bf16, tag="mask", name=f"mask{t}"
                )

            def load_mask_piece(mt_, t, piece):
                c0 = 2048 * piece
                nc.sync.dma_start(
                    mt_[:, c0 : c0 + 2048],
                    maskt[P * t : P * (t + 1), c0 : c0 + 2048],
                )

            # =================================================================
            # startup (all DMAs in consumption order; PE covered by warmup)
            # =================================================================
            wk_sb = load_w(wk, "k")
            kT_sl = [[None] * 4 for _ in range(MC)]
            for st in (0, 1):
                for m in range(MC):
                    kT_sl[m][st] = load_slice(kT, m, st, "kT", f"k{m}_{st}")
            wq_sb = load_w(wq, "q")
            qT_sl = [[None] * 4 for _ in range(MC)]
            for m in range(MC):
                qT_sl[m][0] = load_slice(qT, m, 0, "qT", f"q{m}_0")
            mts = {0: load_mask(0)}
            load_mask_piece(mts[0], 0, 0)

            KT_sb = [None, None]
            KT_sb[0] = qkt_pool.tile([P, S], bf16, tag="KT", name="KT0")
            for st in (0, 1):
                proj_unit(
                    [kT_sl[m][st] for m in range(MC)], wk_sb, 0, KT_sb[0], st,
                    f"k0_{st}",
                )
            for st in (2, 3):
                for m in range(MC):
                    kT_sl[m][st] = load_slice(kT, m, st, "kT", f"k{m}_{st}")
            load_mask_piece(mts[0], 0, 1)
            wv_sb = load_w(wv, "v")
            vT_sl = [[None] * 4 for _ in range(MC)]
            for m in range(MC):
                vT_sl[m][0] = load_slice(vT, m, 0, "vT", f"v{m}_0")
            for st in (2, 3):
                proj_unit(
                    [kT_sl[m][st] for m in range(MC)], wk_sb, 0, KT_sb[0], st,
                    f"k0_{st}",
                )
            QT_sb = [None, None]
            QT_sb[0] = qkt_pool.tile([P, S], bf16, tag="QT", name="QT0")
            proj_unit([qT_sl[m][0] for m in range(MC)], wq_sb, 0, QT_sb[0], 0, "q0_0")
            load_mask_piece(mts[0], 0, 2)
            load_mask_piece(mts[0], 0, 3)
            vext = [None] * KC
            for st in range(4):
                vext[st] = v_unit(
                    [vT_sl[m][0] for m in range(MC)], st, wv_sb, st
                )
            wo_sb = load_w(wo, "o")

            # =================================================================
            # filler schedule: slot index -> list of closures (emitted there)
            # =================================================================
            SLOTS = 8 * 8
            filler = [[] for _ in range(SLOTS + 1)]

            def sched(slot, fn):
                filler[min(max(slot, 0), SLOTS)].append(fn)

            # vT slice DMAs + V units 4..15 (vext[st] first used at slot st//2+3)
            def mk_vdma(stq):
                def f():
                    for m in range(MC):
                        vT_sl[m][stq] = load_slice(vT, m, stq, "vT", f"v{m}_{stq}")
                return f

            def mk_vunit(st):
                def f():
                    stq, soff = divmod(st, 4)
                    vext[st] = v_unit(
                        [vT_sl[m][stq] for m in range(MC)], soff, wv_sb, st
                    )
                return f

            sched(0, mk_vdma(1))
            sched(0, mk_vdma(2))
            sched(2, mk_vdma(3))
            for st in range(4, KC):
                sched((st - 4) // 2, mk_vunit(st))

            # Q proj: (0,t) at slots 2/6/10; (1,t) at 25/30/35/40 with reloads
            def mk_qdma(t, sl, nm):
                def f():
                    for m in range(MC):
                        sl[m][t] = load_slice(qT, m, t, "qT", f"q{nm}{m}_{t}")
                return f

            def mk_qunit(pair, t, sl, nm):
                def f():
                    if QT_sb[pair] is None:
                        QT_sb[pair] = qkt_pool.tile(
                            [P, S], bf16, tag="QT", name="QT1"
                        )
                    proj_unit(
                        [sl[m][t] for m in range(MC)], wq_sb, pair,
                        QT_sb[pair], t, nm,
                    )
                return f

            for t in range(1, 4):
                sched(4 * t - 3, mk_qdma(t, qT_sl, "a"))
                sched(4 * t - 2, mk_qunit(0, t, qT_sl, f"q0_{t}"))
            qT_sl2 = [[None] * 4 for _ in range(MC)]
            for t in range(4):
                sched(18 + 5 * t, mk_qdma(t, qT_sl2, "b"))
                sched(25 + 5 * t, mk_qunit(1, t, qT_sl2, f"q1_{t}"))

            # K proj pair1: kT reload at slots 7-10, units at 12-18
            kT_sl2 = [[None] * 4 for _ in range(MC)]

            def mk_kdma2(st):
                def f():
                    for m in range(MC):
                        kT_sl2[m][st] = load_slice(kT, m, st, "kT", f"k2{m}_{st}")
                return f

            def mk_kunit2(st):
                def f():
                    if KT_sb[1] is None:
                        KT_sb[1] = qkt_pool.tile(
                            [P, S], bf16, tag="KT", name="KT1"
                        )
                    proj_unit(
                        [kT_sl2[m][st] for m in range(MC)], wk_sb, 1,
                        KT_sb[1], st, f"k1_{st}",
                    )
                return f

            for st in range(4):
                sched(7 + st, mk_kdma2(st))
                sched(12 + 2 * st, mk_kunit2(st))

            # mask tiles 1-3 (4 pieces each)
            def mk_mdma(t, piece):
                def f():
                    if t not in mts:
                        mts[t] = load_mask(t)
                    load_mask_piece(mts[t], t, piece)
                return f

            for t in range(1, 4):
                for piece in range(4):
                    sched(8 * t - 7 + piece, mk_mdma(t, piece))

            # =================================================================
            # attention slot pipeline
            # =================================================================
            ones_lhs = sm_pool.tile([DK + 1, P], bf16, tag="ones")
            nc.vector.memset(ones_lhs[:], 1.0)

            DCS = [0, 2, 4, 6, 1, 3, 5, 7]

            def do_readback(t, pairs=(0, 1)):
                ctxg = []
                for p in pairs:
                    cg = ctxg_pool.tile(
                        [P, 4 * QTS], bf16, tag="ctxg", name=f"cg{t}_{p}"
                    )
                    src3 = cc_out[t][p].rearrange("(i pp) q -> pp i q", pp=P)
                    dst3 = cg[:].rearrange("pp (i q) -> pp i q", q=QTS)
                    nc.sync.dma_start(dst3, src3)
                    ctxg.append(cg)
                return ctxg

            def outproj_steps(t, ctxg):
                state = {}

                def unit(qs, i0):
                    if qs not in state:
                        state[qs] = ps_out.tile(
                            [P, DL], fp32, tag="out", name=f"op{t}_{qs}"
                        )
                    op = state[qs]
                    for i in (i0, i0 + 1):
                        dc = DCS[i]
                        src = ctxg[dc % 2][
                            :,
                            QTS * (dc // 2) + P * qs : QTS * (dc // 2)
                            + P * (qs + 1),
                        ]
                        nc.tensor.matmul(
                            op[:],
                            src,
                            wo_sb[dc][:],
                            start=(i == 0),
                            stop=(i == MC - 1),
                        )
                    if i0 + 2 == MC:
                        ys = y_pool.tile(
                            [P, DL], fp32, tag="ysb", name=f"ys{t}_{qs}"
                        )
                        nc.vector.tensor_copy(ys[:], op[:])
                        r = QTS * t + P * qs
                        nc.sync.dma_start(y[r : r + P, :], ys[:])

                for qs in range(4):
                    for i0 in (0, 2, 4, 6):
                        yield lambda qs=qs, i0=i0: unit(qs, i0)

            # ---- flat slot pipeline over (u, grp) ---------------------------
            ATD = 8
            at_store = {}
            cp_store = {}
            rolling_cols = ATD * QTS

            def emit_scores(u, grp):
                pair, t = U_LIST[u]
                if grp == 0:
                    at_store[u] = {
                        h01: attn_pool.tile(
                            [P, rolling_cols], bf16, tag="attn",
                            name=f"at{u}_{h01}",
                        )
                        for h01 in range(2)
                    }
                at = at_store[u]
                mt = mts[t]
                sp = {}
                for h01 in range(2):
                    sp[h01] = ps_big.tile(
                        [P, 1024], fp32, tag="big", name=f"sp{u}_{grp}_{h01}"
                    )
                for j in range(2):
                    kc = 2 * grp + j
                    for h01 in range(2):
                        r0 = DK * h01
                        nc.tensor.matmul(
                            sp[h01][:, QTS * j : QTS * (j + 1)],
                            KT_sb[pair][r0 : r0 + DK, P * kc : P * (kc + 1)],
                            QT_sb[pair][r0 : r0 + DK, QTS * t : QTS * (t + 1)],
                            start=True,
                            stop=True,
                            tile_position=(r0, 0),
                        )
                roff = (2 * grp % ATD) * QTS
                rsl = slice(roff, roff + 1024)
                gsl = slice(1024 * grp, 1024 * (grp + 1))
                for h01 in range(2):
                    nc.scalar.activation(
                        at[h01][:, rsl],
                        sp[h01][:],
                        mybir.ActivationFunctionType.Exp,
                    )
                    nc.vector.tensor_mul(at[h01][:, rsl], at[h01][:, rsl], mt[:, gsl])

            def emit_ctx(u, grp):
                pair, t = U_LIST[u]
                if grp == 0:
                    cp_store[u] = {
                        h01: ps_acc.tile(
                            [P, QTS], fp32, tag="acc", name=f"cp{u}_{h01}"
                        )
                        for h01 in range(2)
                    }
                at = at_store[u]
                cp = cp_store[u]
                for j in range(2):
                    kc = 2 * grp + j
                    roff = (kc % ATD) * QTS
                    for h01 in range(2):
                        h = 2 * pair + h01
                        nc.tensor.matmul(
                            cp[h01][0 : DK + 1, :],
                            vext[kc][:, 65 * h : 65 * h + DK + 1],
                            at[h01][:, roff : roff + QTS],
                            start=(kc == 0),
                            stop=(kc == KC - 1),
                        )

            op_queue = []  # (ready_slot, fn) -- 4-unit (one qs) groups only

            def emit_norm(u, slot):
                pair, t = U_LIST[u]
                cp = cp_store[u]
                for h01 in range(2):
                    srow = sm_pool.tile(
                        [DK + 1, QTS], bf16, tag="srow", name=f"srow{u}_{h01}"
                    )
                    nc.vector.tensor_copy(
                        srow[DK : DK + 1, :], cp[h01][DK : DK + 1, :]
                    )
                    bc = ps_out.tile(
                        [P, QTS], fp32, tag="out", name=f"bc{u}_{h01}"
                    )
                    nc.tensor.matmul(
                        bc[:],
                        ones_lhs[DK : DK + 1, :],
                        srow[DK : DK + 1, :],
                        start=True,
                        stop=True,
                        tile_position=(DK, 0),
                    )
                    recipb = sm_pool.tile(
                        [P, QTS], fp32, tag="recipb", name=f"recipb{u}_{h01}"
                    )
                    nc.vector.reciprocal_approx_fast(out=recipb[:], in_=bc[:])
                    cn = ctxn_pool.tile(
                        [DK, QTS], bf16, tag="ctxn", name=f"cn{u}_{h01}"
                    )
                    nc.vector.tensor_mul(
                        cn[:], cp[h01][0:DK, :], recipb[0:DK, :]
                    )
                    nc.sync.dma_start(
                        cc_in[t][pair][DK * h01 : DK * (h01 + 1), :], cn[:]
                    )
                nc.gpsimd.collective_compute(
                    "AllGather",
                    mybir.AluOpType.bypass,
                    replica_groups=GROUPS,
                    ins=[cc_in[t][pair][:]],
                    outs=[cc_out[t][pair][:]],
                )
                del cp_store[u], at_store[u]

            NSLOT = SLOTS
            ctx_done = 0  # flat index of next ctx slot to emit

            def emit_ctx_flat(lag, slot):
                ul, gl = divmod(lag, 8)
                emit_ctx(ul, gl)
                if gl == 7:
                    emit_norm(ul, slot)
                    pl, tl = U_LIST[ul]
                    if pl == 1 and tl < QT_N - 1:
                        ctxg_t = do_readback(tl)
                        # drain 3+ slots later: the readback DMA waits on the
                        # AllGather; draining sooner head-of-line-blocks PE
                        for k, st_ in enumerate(outproj_steps(tl, ctxg_t)):
                            op_queue.append((slot + 3 + k // 4, st_))

            for i in range(NSLOT):
                u, grp = divmod(i, 8)
                target = i - 3 if i < NSLOT - 8 else i - 1
                while ctx_done <= target and ctx_done < NSLOT:
                    emit_ctx_flat(ctx_done, i)
                    ctx_done += 1
                emit_scores(u, grp)
                ndrain = 0
                while op_queue and op_queue[0][0] <= i and ndrain < 8:
                    op_queue.pop(0)[1]()
                    ndrain += 1
                for fn in filler[i]:
                    fn()
            while ctx_done < NSLOT:
                emit_ctx_flat(ctx_done, NSLOT)
                ctx_done += 1
            for fns in filler[NSLOT]:
                fns()
            ctxg3 = do_readback(QT_N - 1)
            for _, st_ in op_queue:
                st_()
            for st_ in outproj_steps(QT_N - 1, ctxg3):
                st_()

    nc.compile()
    return nc


def _get_nc():
    if "nc" not in _cached:
        _cached["nc"] = _build()
    return _cached["nc"]


def _shard_inputs(q, k, v, mask, w_q, w_k, w_v, w_o):
    in_maps = []
    scale = 1.0 / np.sqrt(DK)
    wqT = (w_q.astype(np.float64) * scale).astype(np.float32).T  # [DM, DM]
    wkT = w_k.T
    wvT = w_v.T
    woT = w_o.T
    mask = np.asarray(mask)
    for c in range(8):
        b, g = c // 4, c % 4
        sl = slice(DL * g, DL * (g + 1))
        mT = np.ascontiguousarray(mask[b].T).astype(BF16)  # [k, q]
        mtiled = np.ascontiguousarray(
            mT.reshape(KC, P, QT_N, QTS).transpose(2, 1, 0, 3)
        ).reshape(QT_N * P, KC * QTS)
        in_maps.append(
            {
                "qT": np.ascontiguousarray(q[b].T).astype(BF16),
                "kT": np.ascontiguousarray(k[b].T).astype(BF16),
                "vT": np.ascontiguousarray(v[b].T).astype(BF16),
                "maskt": mtiled,
                "wq": np.ascontiguousarray(wqT[:, sl]).astype(BF16),
                "wk": np.ascontiguousarray(wkT[:, sl]).astype(BF16),
                "wv": np.ascontiguousarray(wvT[:, sl]).astype(BF16),
                "wo": np.ascontiguousarray(woT[:, sl]).astype(BF16),
            }
        )
    return in_maps


def kernel(q, k, v, mask, w_q, w_k, w_v, w_o, _trace=False, _tmpdir=None):
    from concourse import bass_utils

    nc = _get_nc()
    in_maps = _shard_inputs(q, k, v, mask, w_q, w_k, w_v, w_o)
    res = bass_utils.run_bass_kernel_spmd(
        nc,
        in_maps,
        core_ids=list(range(8)),
        trace=_trace,
        tmpdir=_tmpdir,
    )
    out = np.empty((B, S, DM), dtype=np.float32)
    for c in range(8):
        b, g = c // 4, c % 4
        out[b, :, DL * g : DL * (g + 1)] = res.results[c]["y"]
    if _trace:
        _cached["last_exec_time_ns"] = res.exec_time_ns
        _cached["last_results"] = res
    return out
